# revision 35
# baseline (speedup 1.0000x reference)
"""MMoE-style CustomizedGateControl kernel for 8x TRN2 NeuronCores.

Data-parallel over the batch dim (16384 -> 8 x 2048). Per core, everything
runs in the transposed [feature, batch] layout so the tower GEMMs need no
transpose and bias+ReLU fuse into the scalar-engine PSUM drain:
  - 12 expert GEMMs + gates as f16 matmuls with the weight chunk stationary
    and batch streaming (N=512), output [e, b] in PSUM
  - drain: ACT relu(psum + per-partition bias) -> f16 SBUF, 1024 cols/op
  - gates [16, b] are broadcast to all 128 partitions via a DRAM round-trip
    DMA (stride-0 source), then the gated combine runs on the vector engine
    as f16 tensor_tensor mult/add into info[t] [e, b]
  - tower MLPs consume info [e, b] directly on the PE
All parameters replicated; no collectives.
"""

import sys

if "/opt/trn_rl_repo" not in sys.path:
    sys.path.insert(0, "/opt/trn_rl_repo")

import numpy as np

import concourse.bacc as bacc
import concourse.mybir as mybir
import concourse.tile as tile
from concourse.bass_utils import run_bass_kernel_spmd

# problem dims
B, D, E, H = 16384, 512, 256, 128
S, K, T = 4, 4, 2
NCORES = 8
BC = B // NCORES          # 2048 batch rows per core
P = 128                   # partitions
KC = D // P               # 4 contraction chunks
NE = S + T * K            # 12 experts
G = S + K                 # 8 gate inputs per task
EC = E // P               # 2 e-chunks per expert
JW = 512                  # batch columns per matmul
NJ = BC // JW             # 4 batch blocks
GOFF = T * G              # 16 gate cols, laid out FIRST in wall
WALL = GOFF + NE * E      # 3088

f32 = mybir.dt.float32
f16 = mybir.dt.float16


def _tasks_of(n):
    """Expert order n: shared 0..3, task0 4..7, task1 8..11 -> (t, gate_idx)."""
    if n < S:
        return [(t, n) for t in range(T)]
    t = (n - S) // K
    return [(t, S + (n - S) % K)]


def _build():
    nc = bacc.Bacc("TRN2", target_bir_lowering=False, debug=False)

    xt_d = nc.dram_tensor("xt", [D, BC], f16, kind="ExternalInput").ap()
    wall_d = nc.dram_tensor("wall", [D, WALL], f16, kind="ExternalInput").ap()
    be_d = nc.dram_tensor("be", [P, NE * EC], f32, kind="ExternalInput").ap()
    tw1_d = nc.dram_tensor("tw1", [T, E, H], f16, kind="ExternalInput").ap()
    tb1_d = nc.dram_tensor("tb1", [H, T], f32, kind="ExternalInput").ap()
    tw2_d = nc.dram_tensor("tw2", [H, T], f16, kind="ExternalInput").ap()
    out_d = nc.dram_tensor("out", [T, BC], f32, kind="ExternalOutput").ap()

    with tile.TileContext(nc) as tc:
        with (
            tc.tile_pool(name="gdram", bufs=1, space="DRAM") as gdram_pool,
            tc.tile_pool(name="const", bufs=1) as const,
            tc.tile_pool(name="expt", bufs=8) as expt_pool,
            tc.tile_pool(name="prod", bufs=4) as prod_pool,
            tc.tile_pool(name="hsb", bufs=4) as hsb_pool,
        ):
            gd = gdram_pool.tile([T * G, BC], f16, tag="gd", name="gd")
            xt_t = [const.tile([P, BC], f16, tag=f"xt{k}", name=f"xt{k}") for k in range(KC)]
            wall_t = [const.tile([P, WALL], f16, tag=f"wall{k}", name=f"wall{k}") for k in range(KC)]
            be = const.tile([P, NE * EC], f32, tag="be", name="be")
            tb1 = const.tile([H, T], f32, tag="tb1", name="tb1")
            tw2 = const.tile([H, T], f16, tag="tw2", name="tw2")
            gsb = const.tile([T * G, BC], f16, tag="gsb", name="gsb")
            grep = {}
            for t in range(T):
                for g in range(G):
                    grep[(t, g)] = const.tile(
                        [P, BC], f16, tag=f"grep{t}_{g}", name=f"grep{t}_{g}"
                    )
            out_sb = const.tile([1, T * BC], f32, tag="out_sb", name="out_sb")
            infoT = {}
            for t in range(T):
                for ec in range(EC):
                    infoT[(t, ec)] = const.tile(
                        [P, BC], f16, tag=f"infoT{t}_{ec}", name=f"infoT{t}_{ec}"
                    )
            tw1_t = {}
            for t in range(T):
                for ec in range(EC):
                    tw1_t[(t, ec)] = const.tile(
                        [P, H], f16, tag=f"tw1_{t}_{ec}", name=f"tw1_{t}_{ec}"
                    )

            # warm-up source tiles (filled by gpsimd, no DMA dependency)
            wu_w = const.tile([P, P], f16, tag="wu_w", name="wu_w")
            wu_x = const.tile([P, JW], f16, tag="wu_x", name="wu_x")
            nc.gpsimd.memset(wu_w[:], 0)
            nc.gpsimd.memset(wu_x[:], 0)

            # ---- input DMA. Per-queue DMA bw is ~115 GB/s, so the early
            # window is ordered to feed the PE exactly in consumption order:
            # xt lands as j-halves (k-major within each half) split over the
            # sync/scalar queues; wall lands in fine chunks on gpsimd ----
            for h in range(2):
                hs_ = slice(h * 1024, (h + 1) * 1024)
                for k in range(KC):
                    rs = slice(k * P, (k + 1) * P)
                    q = nc.sync if k < 2 else nc.scalar
                    q.dma_start(xt_t[k][:, hs_], xt_d[rs, hs_])
                if h == 0:
                    nc.scalar.dma_start(be[:], be_d[:])
            # gpsimd: wall chunks sized to expert consumption order:
            # [gates+e0 | e1 | e2,e3 | task0 | task1], k-major within each
            WA = GOFF + S * E          # 1040
            for c0, c1 in [(0, GOFF + E), (GOFF + E, GOFF + 2 * E),
                           (GOFF + 2 * E, WA), (WA, WA + K * E), (WA + K * E, WALL)]:
                for k in range(KC):
                    rs = slice(k * P, (k + 1) * P)
                    nc.gpsimd.dma_start(wall_t[k][:, c0:c1], wall_d[rs, c0:c1])
            # scalar: tiny tower consts after its xt chunks, long before use
            for t in range(T):
                for ec in range(EC):
                    nc.scalar.dma_start(
                        tw1_t[(t, ec)][:], tw1_d[t, ec * P : (ec + 1) * P, :]
                    )
            nc.scalar.dma_start(tb1[:], tb1_d[:])
            nc.scalar.dma_start(tw2[:], tw2_d[:])

            with (
                tc.tile_pool(name="expps", bufs=2, space="PSUM") as expps_pool,
                tc.tile_pool(name="hps", bufs=4, space="PSUM") as hps_pool,
            ):
                # ---- PE warm-up: dependency-free matmuls bridge the DMA
                # wait so HAM un-throttles before the real sweep starts ----
                wu_ps = hps_pool.tile([P, JW], f32, tag="hps", name="wups")
                for r in range(13):
                    nc.tensor.matmul(wu_ps[:], wu_w[:], wu_x[:], start=True, stop=True)

                info_init = set()

                def emit_gate(j):
                    js = slice(j * JW, (j + 1) * JW)
                    gp = hps_pool.tile([T * G, JW], f32, tag="hps", name="gps")
                    for k in range(KC):
                        nc.tensor.matmul(
                            gp[:],
                            wall_t[k][:, 0:GOFF],
                            xt_t[k][:, js],
                            start=(k == 0),
                            stop=(k == KC - 1),
                        )
                    nc.scalar.copy(gsb[:, js], gp[:])

                def emit_exp(n, ec, expT, jps):
                    c0 = GOFF + n * E + ec * P
                    for jp in jps:
                        pe = expps_pool.tile([P, 2 * JW], f32, tag="expps", name="expps")
                        for j2 in range(2):
                            js = slice((jp * 2 + j2) * JW, (jp * 2 + j2 + 1) * JW)
                            for k in range(KC):
                                nc.tensor.matmul(
                                    pe[:, j2 * JW : (j2 + 1) * JW],
                                    wall_t[k][:, c0 : c0 + P],
                                    xt_t[k][:, js],
                                    start=(k == 0),
                                    stop=(k == KC - 1),
                                )
                        nc.scalar.activation(
                            expT[:, jp * 2 * JW : (jp + 1) * 2 * JW],
                            pe[:],
                            mybir.ActivationFunctionType.Relu,
                            bias=be[:, n * EC + ec : n * EC + ec + 1],
                        )

                def emit_combine(n, ec, expT):
                    for t, g in _tasks_of(n):
                        if (t, ec) not in info_init:
                            info_init.add((t, ec))
                            nc.vector.tensor_mul(
                                infoT[(t, ec)][:], expT[:], grep[(t, g)][:]
                            )
                        elif n == S + K - 1:
                            # last expert of this task: split by halves so
                            # the tower's first j-blocks unblock earlier
                            pr = prod_pool.tile([P, BC], f16, tag="prod", name="prod")
                            for h in range(2):
                                hs_ = slice(h * (BC // 2), (h + 1) * (BC // 2))
                                nc.vector.tensor_mul(
                                    pr[:, hs_], expT[:, hs_], grep[(t, g)][:, hs_]
                                )
                                nc.vector.tensor_add(
                                    infoT[(t, ec)][:, hs_],
                                    infoT[(t, ec)][:, hs_],
                                    pr[:, hs_],
                                )
                        else:
                            pr = prod_pool.tile([P, BC], f16, tag="prod", name="prod")
                            nc.vector.tensor_mul(pr[:], expT[:], grep[(t, g)][:])
                            nc.vector.tensor_add(
                                infoT[(t, ec)][:], infoT[(t, ec)][:], pr[:]
                            )

                # ---- prologue: interleave gates and the first two experts
                # so PE work tracks the DMA arrival order (xt h0 -> h1) ----
                emit_gate(0)
                emit_gate(1)
                first4 = [(0, 0), (0, 1), (1, 0), (1, 1)]
                eT = {}
                for n, ec in first4:
                    eT[(n, ec)] = expt_pool.tile([P, BC], f16, tag="expt", name="expt")
                    emit_exp(n, ec, eT[(n, ec)], [0])
                emit_gate(2)
                emit_gate(3)
                nc.sync.dma_start(gd[:], gsb[:])
                # broadcast each gate row to all 128 partitions. sync carries
                # the time-critical shared-expert rows in consumption order;
                # gpsimd (busy with wall until ~20us) carries task rows,
                # which aren't consumed until much later.
                for g in range(S):
                    for t in range(T):
                        nc.sync.dma_start(
                            grep[(t, g)][:],
                            gd[t * G + g : t * G + g + 1, :].broadcast_to([P, BC]),
                        )
                for t in range(T):
                    for g in range(S, G):
                        nc.gpsimd.dma_start(
                            grep[(t, g)][:],
                            gd[t * G + g : t * G + g + 1, :].broadcast_to([P, BC]),
                        )
                for n, ec in first4:
                    emit_exp(n, ec, eT[(n, ec)], [1])
                    emit_combine(n, ec, eT[(n, ec)])

                def emit_tower(t):
                    # phase 1: all tower-1 matmuls back-to-back; ACT drains
                    # trail behind. phase 2: tower-2 matmuls, each waiting
                    # only an already-finished drain.
                    hs_t = []
                    for j in range(NJ):
                        js = slice(j * JW, (j + 1) * JW)
                        hp = hps_pool.tile([P, JW], f32, tag="hps", name="hps")
                        for ec in range(EC):
                            nc.tensor.matmul(
                                hp[:],
                                tw1_t[(t, ec)][:],
                                infoT[(t, ec)][:, js],
                                start=(ec == 0),
                                stop=(ec == EC - 1),
                            )
                        hs = hsb_pool.tile([P, JW], f16, tag="hsb", name="hsb")
                        if t == 1 and j % 2 == 1:
                            # final tower: split drains across ACT and DVE so
                            # the tower-2 matmuls unblock ~2x faster
                            nc.vector.tensor_scalar(
                                hs[:], hp[:], tb1[:, t : t + 1], 0.0,
                                op0=mybir.AluOpType.add, op1=mybir.AluOpType.max,
                            )
                        else:
                            nc.scalar.activation(
                                hs[:],
                                hp[:],
                                mybir.ActivationFunctionType.Relu,
                                bias=tb1[:, t : t + 1],
                            )
                        hs_t.append(hs)
                    for j in range(NJ):
                        op = hps_pool.tile([T * G, JW], f32, tag="hps", name="ops")
                        nc.tensor.matmul(
                            op[0:1, :], tw2[:, t : t + 1], hs_t[j][:], start=True, stop=True
                        )
                        r0 = (t * NJ + j) * JW
                        nc.vector.tensor_copy(out_sb[0:1, r0 : r0 + JW], op[0:1, :])
                        nc.sync.dma_start(
                            out_d.rearrange("t n -> (t n)")[None, r0 : r0 + JW],
                            out_sb[0:1, r0 : r0 + JW],
                        )

                for n in range(2, NE - 1):
                    for ec in range(EC):
                        expT = expt_pool.tile([P, BC], f16, tag="expt", name="expt")
                        emit_exp(n, ec, expT, [0, 1])
                        emit_combine(n, ec, expT)
                    if n == 9:
                        emit_tower(0)

                # last expert: interleave its two e-chunks and combine in
                # half-batch steps so tower-1 for task 1 unblocks j-block by
                # j-block instead of waiting for the full combine
                t_, g_ = _tasks_of(NE - 1)[0]
                eL = {}
                for ec in range(EC):
                    eL[ec] = expt_pool.tile([P, BC], f16, tag="expt", name="expt")
                for jp in range(2):
                    for ec in range(EC):
                        emit_exp(NE - 1, ec, eL[ec], [jp])
                    for ec in range(EC):
                        hs_ = slice(jp * 1024, (jp + 1) * 1024)
                        pr = prod_pool.tile([P, BC], f16, tag="prod", name="prod")
                        nc.vector.tensor_mul(
                            pr[:, hs_], eL[ec][:, hs_], grep[(t_, g_)][:, hs_]
                        )
                        nc.vector.tensor_add(
                            infoT[(t_, ec)][:, hs_],
                            infoT[(t_, ec)][:, hs_],
                            pr[:, hs_],
                        )
                emit_tower(1)

    nc.compile()
    return nc


_NC = None


def _get_nc():
    global _NC
    if _NC is None:
        _NC = _build()
    return _NC


def _prep_shared(shared_W, shared_b, task_W, task_b, gate_W, tower_W1, tower_b1, tower_W2):
    cols = [np.asarray(gate_W[t]) for t in range(T)]  # gate col t*G+g first
    cols += [np.asarray(shared_W[s]) for s in range(S)]
    cols += [np.asarray(task_W[t, k]) for t in range(T) for k in range(K)]
    wall = np.ascontiguousarray(np.concatenate(cols, axis=1), dtype=np.float16)
    bias_all = np.concatenate(
        [np.asarray(shared_b).reshape(-1), np.asarray(task_b).reshape(-1)]
    ).astype(np.float32)
    # be column n*EC+ec = bias of expert n, e-chunk ec, as a per-partition vec
    be = np.ascontiguousarray(bias_all.reshape(NE * EC, P).T.astype(np.float32))
    tw1 = np.ascontiguousarray(tower_W1, dtype=np.float16)
    tb1 = np.ascontiguousarray(np.asarray(tower_b1).T, dtype=np.float32)   # [H, T]
    tw2 = np.ascontiguousarray(np.asarray(tower_W2)[:, :, 0].T, dtype=np.float16)  # [H, T]
    return wall, be, tw1, tb1, tw2


def kernel(
    x,
    shared_W,
    shared_b,
    task_W,
    task_b,
    gate_W,
    tower_W1,
    tower_b1,
    tower_W2,
    tower_b2,
    _trace=False,
    _tmpdir=None,
):
    nc = _get_nc()
    x = np.asarray(x, dtype=np.float32)
    wall, be, tw1, tb1, tw2 = _prep_shared(
        shared_W, shared_b, task_W, task_b, gate_W, tower_W1, tower_b1, tower_W2
    )
    in_maps = []
    for c in range(NCORES):
        xt = np.ascontiguousarray(x[c * BC : (c + 1) * BC, :].T.astype(np.float16))
        in_maps.append(
            {
                "xt": xt,
                "wall": wall,
                "be": be,
                "tw1": tw1,
                "tb1": tb1,
                "tw2": tw2,
            }
        )
    kw = {}
    if _trace:
        kw = {"trace": True, "tmpdir": _tmpdir}
    res = run_bass_kernel_spmd(nc, in_maps, core_ids=list(range(NCORES)), **kw)
    out = np.concatenate([res.results[c]["out"] for c in range(NCORES)], axis=1)
    out = out + np.asarray(tower_b2, dtype=np.float32)[:, 0][:, None]
    result = out[:, :, None].astype(np.float32)  # [T, B, 1]
    if _trace:
        return result, res
    return result


# revision 40
# speedup vs baseline: 1.0364x; 1.0364x over previous
"""MMoE-style CustomizedGateControl kernel for 8x TRN2 NeuronCores.

Data-parallel over the batch dim (16384 -> 8 x 2048). Per core, everything
runs in the transposed [feature, batch] layout so the tower GEMMs need no
transpose and bias+ReLU fuse into the scalar-engine PSUM drain:
  - 12 expert GEMMs + gates as f16 matmuls with the weight chunk stationary
    and batch streaming (N=512), output [e, b] in PSUM
  - drain: ACT relu(psum + per-partition bias) -> f16 SBUF, 1024 cols/op
  - gates [16, b] are broadcast to all 128 partitions via a DRAM round-trip
    DMA (stride-0 source), then the gated combine runs on the vector engine
    as f16 tensor_tensor mult/add into info[t] [e, b]
  - tower MLPs consume info [e, b] directly on the PE
All parameters replicated; no collectives.
"""

import sys

if "/opt/trn_rl_repo" not in sys.path:
    sys.path.insert(0, "/opt/trn_rl_repo")

import numpy as np

import concourse.bacc as bacc
import concourse.mybir as mybir
import concourse.tile as tile
from concourse.bass_utils import run_bass_kernel_spmd

# problem dims
B, D, E, H = 16384, 512, 256, 128
S, K, T = 4, 4, 2
NCORES = 8
BC = B // NCORES          # 2048 batch rows per core
P = 128                   # partitions
KC = D // P               # 4 contraction chunks
NE = S + T * K            # 12 experts
G = S + K                 # 8 gate inputs per task
EC = E // P               # 2 e-chunks per expert
JW = 512                  # batch columns per matmul
NJ = BC // JW             # 4 batch blocks
GOFF = T * G              # 16 gate cols, laid out FIRST in wall
WALL = GOFF + NE * E      # 3088

f32 = mybir.dt.float32
f16 = mybir.dt.float16


def _tasks_of(n):
    """Expert order n: shared 0..3, task0 4..7, task1 8..11 -> (t, gate_idx)."""
    if n < S:
        return [(t, n) for t in range(T)]
    t = (n - S) // K
    return [(t, S + (n - S) % K)]


def _build():
    nc = bacc.Bacc("TRN2", target_bir_lowering=False, debug=False)

    xt_d = nc.dram_tensor("xt", [D, BC], f16, kind="ExternalInput").ap()
    wall_d = nc.dram_tensor("wall", [D, WALL], f16, kind="ExternalInput").ap()
    be_d = nc.dram_tensor("be", [P, NE * EC], f32, kind="ExternalInput").ap()
    tw1_d = nc.dram_tensor("tw1", [T, E, H], f16, kind="ExternalInput").ap()
    tb1_d = nc.dram_tensor("tb1", [H, T], f32, kind="ExternalInput").ap()
    tw2_d = nc.dram_tensor("tw2", [H, T], f16, kind="ExternalInput").ap()
    out_d = nc.dram_tensor("out", [T, BC], f32, kind="ExternalOutput").ap()

    with tile.TileContext(nc) as tc:
        with (
            tc.tile_pool(name="gdram", bufs=1, space="DRAM") as gdram_pool,
            tc.tile_pool(name="const", bufs=1) as const,
            tc.tile_pool(name="expt", bufs=8) as expt_pool,
            tc.tile_pool(name="prod", bufs=4) as prod_pool,
            tc.tile_pool(name="hsb", bufs=4) as hsb_pool,
        ):
            gd = gdram_pool.tile([T * G, BC], f16, tag="gd", name="gd")
            xt_t = [const.tile([P, BC], f16, tag=f"xt{k}", name=f"xt{k}") for k in range(KC)]
            wall_t = [const.tile([P, WALL], f16, tag=f"wall{k}", name=f"wall{k}") for k in range(KC)]
            be = const.tile([P, NE * EC], f32, tag="be", name="be")
            tb1 = const.tile([H, T], f32, tag="tb1", name="tb1")
            tw2 = const.tile([H, T], f16, tag="tw2", name="tw2")
            gsb = const.tile([T * G, BC], f16, tag="gsb", name="gsb")
            grep = {}
            for t in range(T):
                for g in range(G):
                    grep[(t, g)] = const.tile(
                        [P, BC], f16, tag=f"grep{t}_{g}", name=f"grep{t}_{g}"
                    )
            out_sb = const.tile([1, T * BC], f32, tag="out_sb", name="out_sb")
            infoT = {}
            for t in range(T):
                for ec in range(EC):
                    infoT[(t, ec)] = const.tile(
                        [P, BC], f16, tag=f"infoT{t}_{ec}", name=f"infoT{t}_{ec}"
                    )
            tw1_t = {}
            for t in range(T):
                for ec in range(EC):
                    tw1_t[(t, ec)] = const.tile(
                        [P, H], f16, tag=f"tw1_{t}_{ec}", name=f"tw1_{t}_{ec}"
                    )

            # warm-up source tiles (filled by gpsimd, no DMA dependency)
            wu_w = const.tile([P, P], f16, tag="wu_w", name="wu_w")
            wu_x = const.tile([P, JW], f16, tag="wu_x", name="wu_x")
            nc.gpsimd.memset(wu_w[:], 0)
            nc.gpsimd.memset(wu_x[:], 0)

            # ---- input DMA. Per-queue DMA bw is ~115 GB/s, so the early
            # window is ordered to feed the PE exactly in consumption order:
            # xt lands as j-halves (k-major within each half) split over the
            # sync/scalar queues; wall lands in fine chunks on gpsimd ----
            nc.sync.dma_start(be[:], be_d[:])
            for h in range(2):
                hs_ = slice(h * 1024, (h + 1) * 1024)
                for k in range(KC):
                    rs = slice(k * P, (k + 1) * P)
                    q = nc.sync if k < 2 else nc.scalar
                    q.dma_start(xt_t[k][:, hs_], xt_d[rs, hs_])
            # gpsimd: wall chunks sized to expert consumption order:
            # [gates+e0 | e1 | e2,e3 | task0 | task1], k-major within each
            WA = GOFF + S * E          # 1040
            for c0, c1 in [(0, GOFF + E), (GOFF + E, GOFF + 2 * E),
                           (GOFF + 2 * E, WA), (WA, WA + K * E), (WA + K * E, WALL)]:
                for k in range(KC):
                    rs = slice(k * P, (k + 1) * P)
                    nc.gpsimd.dma_start(wall_t[k][:, c0:c1], wall_d[rs, c0:c1])
            # scalar: tiny tower consts after its xt chunks, long before use
            for t in range(T):
                for ec in range(EC):
                    nc.scalar.dma_start(
                        tw1_t[(t, ec)][:], tw1_d[t, ec * P : (ec + 1) * P, :]
                    )
            nc.scalar.dma_start(tb1[:], tb1_d[:])
            nc.scalar.dma_start(tw2[:], tw2_d[:])

            with (
                tc.tile_pool(name="expps", bufs=2, space="PSUM") as expps_pool,
                tc.tile_pool(name="hps", bufs=4, space="PSUM") as hps_pool,
            ):
                # ---- PE warm-up: dependency-free matmuls bridge the DMA
                # wait so HAM un-throttles before the real sweep starts ----
                wu_ps = hps_pool.tile([P, JW], f32, tag="hps", name="wups")
                for r in range(16):
                    nc.tensor.matmul(wu_ps[:], wu_w[:], wu_x[:], start=True, stop=True)

                info_init = set()

                def emit_gate(j):
                    js = slice(j * JW, (j + 1) * JW)
                    gp = hps_pool.tile([T * G, JW], f32, tag="hps", name="gps")
                    for k in range(KC):
                        nc.tensor.matmul(
                            gp[:],
                            wall_t[k][:, 0:GOFF],
                            xt_t[k][:, js],
                            start=(k == 0),
                            stop=(k == KC - 1),
                        )
                    nc.scalar.copy(gsb[:, js], gp[:])

                def emit_exp(n, ec, expT, jps):
                    c0 = GOFF + n * E + ec * P
                    for jp in jps:
                        pe = expps_pool.tile([P, 2 * JW], f32, tag="expps", name="expps")
                        for j2 in range(2):
                            js = slice((jp * 2 + j2) * JW, (jp * 2 + j2 + 1) * JW)
                            for k in range(KC):
                                nc.tensor.matmul(
                                    pe[:, j2 * JW : (j2 + 1) * JW],
                                    wall_t[k][:, c0 : c0 + P],
                                    xt_t[k][:, js],
                                    start=(k == 0),
                                    stop=(k == KC - 1),
                                )
                        nc.scalar.activation(
                            expT[:, jp * 2 * JW : (jp + 1) * 2 * JW],
                            pe[:],
                            mybir.ActivationFunctionType.Relu,
                            bias=be[:, n * EC + ec : n * EC + ec + 1],
                        )

                def emit_combine(n, ec, expT):
                    for t, g in _tasks_of(n):
                        if (t, ec) not in info_init:
                            info_init.add((t, ec))
                            nc.vector.tensor_mul(
                                infoT[(t, ec)][:], expT[:], grep[(t, g)][:]
                            )
                        elif n in (S + K - 1, NE - 1):
                            # last expert of this task: split by halves so
                            # the tower's first j-blocks unblock earlier
                            pr = prod_pool.tile([P, BC], f16, tag="prod", name="prod")
                            for h in range(2):
                                hs_ = slice(h * (BC // 2), (h + 1) * (BC // 2))
                                nc.vector.tensor_mul(
                                    pr[:, hs_], expT[:, hs_], grep[(t, g)][:, hs_]
                                )
                                nc.vector.tensor_add(
                                    infoT[(t, ec)][:, hs_],
                                    infoT[(t, ec)][:, hs_],
                                    pr[:, hs_],
                                )
                        else:
                            pr = prod_pool.tile([P, BC], f16, tag="prod", name="prod")
                            nc.vector.tensor_mul(pr[:], expT[:], grep[(t, g)][:])
                            nc.vector.tensor_add(
                                infoT[(t, ec)][:], infoT[(t, ec)][:], pr[:]
                            )

                # ---- prologue: interleave gates and the first two experts
                # so PE work tracks the DMA arrival order (xt h0 -> h1) ----
                emit_gate(0)
                emit_gate(1)
                first4 = [(0, 0), (0, 1), (1, 0), (1, 1)]
                eT = {}
                for n, ec in first4:
                    eT[(n, ec)] = expt_pool.tile([P, BC], f16, tag="expt", name="expt")
                    emit_exp(n, ec, eT[(n, ec)], [0])
                emit_gate(2)
                emit_gate(3)
                nc.sync.dma_start(gd[:], gsb[:])
                # broadcast each gate row to all 128 partitions. sync carries
                # the time-critical shared-expert rows in consumption order;
                # gpsimd (busy with wall until ~20us) carries task rows,
                # which aren't consumed until much later.
                for g in range(S):
                    for t in range(T):
                        nc.sync.dma_start(
                            grep[(t, g)][:],
                            gd[t * G + g : t * G + g + 1, :].broadcast_to([P, BC]),
                        )
                for t in range(T):
                    for g in range(S, G):
                        nc.gpsimd.dma_start(
                            grep[(t, g)][:],
                            gd[t * G + g : t * G + g + 1, :].broadcast_to([P, BC]),
                        )
                for n, ec in first4:
                    emit_exp(n, ec, eT[(n, ec)], [1])
                    emit_combine(n, ec, eT[(n, ec)])

                def emit_tower(t):
                    # phase 1: all tower-1 matmuls back-to-back; ACT drains
                    # trail behind. phase 2: tower-2 matmuls, each waiting
                    # only an already-finished drain.
                    hs_t = []
                    for j in range(NJ):
                        js = slice(j * JW, (j + 1) * JW)
                        hp = hps_pool.tile([P, JW], f32, tag="hps", name="hps")
                        for ec in range(EC):
                            nc.tensor.matmul(
                                hp[:],
                                tw1_t[(t, ec)][:],
                                infoT[(t, ec)][:, js],
                                start=(ec == 0),
                                stop=(ec == EC - 1),
                            )
                        hs = hsb_pool.tile([P, JW], f16, tag="hsb", name="hsb")
                        nc.scalar.activation(
                            hs[:],
                            hp[:],
                            mybir.ActivationFunctionType.Relu,
                            bias=tb1[:, t : t + 1],
                        )
                        hs_t.append(hs)
                    for j in range(NJ):
                        op = hps_pool.tile([T * G, JW], f32, tag="hps", name="ops")
                        nc.tensor.matmul(
                            op[0:1, :], tw2[:, t : t + 1], hs_t[j][:], start=True, stop=True
                        )
                        r0 = (t * NJ + j) * JW
                        nc.vector.tensor_copy(out_sb[0:1, r0 : r0 + JW], op[0:1, :])
                        nc.sync.dma_start(
                            out_d.rearrange("t n -> (t n)")[None, r0 : r0 + JW],
                            out_sb[0:1, r0 : r0 + JW],
                        )

                for n in range(2, NE):
                    for ec in range(EC):
                        expT = expt_pool.tile([P, BC], f16, tag="expt", name="expt")
                        emit_exp(n, ec, expT, [0, 1])
                        emit_combine(n, ec, expT)
                    if n == 9:
                        emit_tower(0)
                    if n == NE - 1:
                        emit_tower(1)

    nc.compile()
    return nc


_NC = None


def _get_nc():
    global _NC
    if _NC is None:
        _NC = _build()
    return _NC


def _prep_shared(shared_W, shared_b, task_W, task_b, gate_W, tower_W1, tower_b1, tower_W2):
    cols = [np.asarray(gate_W[t]) for t in range(T)]  # gate col t*G+g first
    cols += [np.asarray(shared_W[s]) for s in range(S)]
    cols += [np.asarray(task_W[t, k]) for t in range(T) for k in range(K)]
    wall = np.ascontiguousarray(np.concatenate(cols, axis=1), dtype=np.float16)
    bias_all = np.concatenate(
        [np.asarray(shared_b).reshape(-1), np.asarray(task_b).reshape(-1)]
    ).astype(np.float32)
    # be column n*EC+ec = bias of expert n, e-chunk ec, as a per-partition vec
    be = np.ascontiguousarray(bias_all.reshape(NE * EC, P).T.astype(np.float32))
    tw1 = np.ascontiguousarray(tower_W1, dtype=np.float16)
    tb1 = np.ascontiguousarray(np.asarray(tower_b1).T, dtype=np.float32)   # [H, T]
    tw2 = np.ascontiguousarray(np.asarray(tower_W2)[:, :, 0].T, dtype=np.float16)  # [H, T]
    return wall, be, tw1, tb1, tw2


def kernel(
    x,
    shared_W,
    shared_b,
    task_W,
    task_b,
    gate_W,
    tower_W1,
    tower_b1,
    tower_W2,
    tower_b2,
    _trace=False,
    _tmpdir=None,
):
    nc = _get_nc()
    x = np.asarray(x, dtype=np.float32)
    wall, be, tw1, tb1, tw2 = _prep_shared(
        shared_W, shared_b, task_W, task_b, gate_W, tower_W1, tower_b1, tower_W2
    )
    in_maps = []
    for c in range(NCORES):
        xt = np.ascontiguousarray(x[c * BC : (c + 1) * BC, :].T.astype(np.float16))
        in_maps.append(
            {
                "xt": xt,
                "wall": wall,
                "be": be,
                "tw1": tw1,
                "tb1": tb1,
                "tw2": tw2,
            }
        )
    kw = {}
    if _trace:
        kw = {"trace": True, "tmpdir": _tmpdir}
    res = run_bass_kernel_spmd(nc, in_maps, core_ids=list(range(NCORES)), **kw)
    out = np.concatenate([res.results[c]["out"] for c in range(NCORES)], axis=1)
    out = out + np.asarray(tower_b2, dtype=np.float32)[:, 0][:, None]
    result = out[:, :, None].astype(np.float32)  # [T, B, 1]
    if _trace:
        return result, res
    return result
